# revision 21
# baseline (speedup 1.0000x reference)
"""AttentionX Trainium2 kernel: 8-way head-parallel attention, v5.

Reference computation (B=1, N=2048, C_Q=256, H=8, C_HID=32):
    q = (q_x @ Wq) * 1/sqrt(32); k = kv_x @ Wk; v = kv_x @ Wv
    scores = q k^T + attn_bias; a = softmax(scores); o = a v
    out = (o * sigmoid(q_x @ Wg)) @ Wo

Sharding: one head per NeuronCore. Host combines: out = sum_h partial_h / sums_h.

The binding constraint on TRN2 is the HAM clock gate: the PE array runs at
1.2GHz unless it is near-continuously busy (then 2.4GHz). PE-array tiling
(tile_position) raises per-matmul concurrency but LOWERS duty cycle, which
keeps the clock throttled and is a net loss. So v5 runs EVERY matmul in
plain 128x128 mode and sizes PE work to ~match the ACT exp floor (~32us):
  - scores q k^T uses host-zero-padded weights (K rows 32:128 = 0), cost is
    free-dim-bound so padding is free;
  - PV uses vhat padded to 65 columns ([v | ones | zeros]) so the output
    partition dim rounds to 128 (full mode); ones column emits softmax
    denominators as o row 32;
  - bias is applied two ways to balance engines: 16 blocks accumulate raw
    bias into score PSUM via eye-matmuls (PE), 48 blocks multiply
    host-precomputed exp(bias) into exp(scores) on DVE at f16 2x rate;
  - exp has bias=-ln(16) for f16 range (cancels in the softmax quotient);
  - sigmoid(x) = 0.5*(1+tanh(x/2)): tanh shares exp's ACT table set, 0.5 is
    folded into Wo, +1 into one tensor_scalar;
  - all 8.4MB of bias slabs are preloaded to SBUF by early DMAs; x inputs
    stream in 4 chunks so the first projection matmul starts ASAP;
  - denominators ride through an augmented Wo column; output is one
    contiguous [128, 16*257] f16 block, reshaped on the host.
"""

import numpy as np

_STATE = {}

B, N, CQ, H, CH = 1, 2048, 256, 8, 32
NKB = N // 128  # 16 k-blocks of 128 keys
NQC = 4  # q-chunks of 512 queries
QC = N // NQC  # 512
HG = 8  # half-groups of 2 k-blocks per q-chunk
HW2 = N // 2  # 1024 score columns per half-group
VW = 65  # padded vhat width: [v(32) | ones(1) | zeros(32)]
LN16 = float(np.log(16.0))


def _is_eye_block(hg, i):
    """Blocks whose bias is PE-eye-accumulated (raw slab); rest are DVE."""
    return hg % 2 == 0 and i == 0


def _build_nc():
    import concourse.bacc as bacc
    import concourse.tile as tile
    from concourse import mybir

    F32 = mybir.dt.float32
    F16 = mybir.dt.float16
    AF = mybir.ActivationFunctionType

    nc = bacc.Bacc("TRN2", target_bir_lowering=False, debug=False, num_devices=H)

    xq_d = nc.dram_tensor("xq", [128, 2 * N], F16, kind="ExternalInput")
    xkv_d = nc.dram_tensor("xkv", [128, 2 * N], F16, kind="ExternalInput")
    wq_d = nc.dram_tensor("wq", [128, 256], F16, kind="ExternalInput")
    wk_d = nc.dram_tensor("wk", [128, 256], F16, kind="ExternalInput")
    wg_d = nc.dram_tensor("wg", [128, 256], F16, kind="ExternalInput")
    wv_d = nc.dram_tensor("wv", [128, 64], F16, kind="ExternalInput")
    wo_d = nc.dram_tensor("wo", [128, 257], F16, kind="ExternalInput")
    eye_d = nc.dram_tensor("eye", [128, 128], F16, kind="ExternalInput")
    eb_d = nc.dram_tensor("eb", [128, 32 * HW2], F16, kind="ExternalInput")
    out_d = nc.dram_tensor("out", [128, 16 * 257], F16, kind="ExternalOutput")

    with tile.TileContext(nc) as tc:
        with (
            tc.tile_pool(name="const", bufs=1) as cpool,
            tc.tile_pool(name="proj", bufs=1) as ppool,
            tc.tile_pool(name="pexp", bufs=3) as pxpool,
            tc.tile_pool(name="pmul", bufs=3) as pmpool,
            tc.tile_pool(name="ogp", bufs=2) as ogpool,
            tc.tile_pool(name="outs", bufs=1) as opool,
        ):
            xq = cpool.tile([128, 2 * N], F16)
            xkv = cpool.tile([128, 2 * N], F16)
            wq = cpool.tile([128, 256], F16)
            nc.sync.dma_start(out=wq, in_=wq_d[:, :])
            wk = cpool.tile([128, 256], F16)
            nc.sync.dma_start(out=wk, in_=wk_d[:, :])
            wg = cpool.tile([128, 256], F16)
            nc.sync.dma_start(out=wg, in_=wg_d[:, :])
            wv = cpool.tile([128, 64], F16)
            nc.sync.dma_start(out=wv, in_=wv_d[:, :])
            wo = cpool.tile([128, 257], F16)
            nc.sync.dma_start(out=wo, in_=wo_d[:, :])
            eye = cpool.tile([128, 128], F16)
            nc.sync.dma_start(out=eye, in_=eye_d[:, :])
            # x inputs in chunks so the first projection matmul starts early
            for f in range(4):
                for src_t, src_d in ((xq, xq_d), (xkv, xkv_d)):
                    for h2 in range(2):
                        sl = slice(N * h2 + QC * f, N * h2 + QC * (f + 1))
                        nc.sync.dma_start(out=src_t[:, sl], in_=src_d[:, sl])
            # bias preload: one big SBUF buffer, 8 chunk DMAs (1MB each)
            ebsb = cpool.tile([128, 32 * HW2], F16)
            for cc in range(8):
                nc.sync.dma_start(
                    out=ebsb[:, 4 * HW2 * cc : 4 * HW2 * (cc + 1)],
                    in_=eb_d[:, 4 * HW2 * cc : 4 * HW2 * (cc + 1)],
                )

            nln16 = cpool.tile([128, 1], F32)
            nc.vector.memset(nln16, -LN16)

            qT4 = ppool.tile([128, N], F16, tag="qT4")
            kT4 = ppool.tile([128, N], F16, tag="kT4")
            gt4 = ppool.tile([128, N], F16, tag="gt4")
            tp1 = ppool.tile([33, N], F16, tag="tp1")
            vhat = ppool.tile([128, NKB * VW], F16, tag="vhat")
            outsb = opool.tile([128, 16 * 257], F16)

            nc.vector.memset(vhat, 0.0)
            for r in range(NKB):
                nc.vector.memset(vhat[:, VW * r + 32 : VW * r + 33], 1.0)
            nc.vector.memset(tp1[32:33, :], 1.0)

            # ---- stage 1: projections ----
            with (
                tc.tile_pool(name="proj_ps", bufs=2, space="PSUM") as proj_ps,
                tc.tile_pool(name="v_ps", bufs=2, space="PSUM") as v_ps,
                nc.named_scope("stage1_proj"),
            ):
                for w, src, dst in ((wq, xq, qT4), (wk, xkv, kT4)):
                    for f in range(4):
                        pp = proj_ps.tile([128, QC], F32, tag="pp")
                        nc.tensor.matmul(
                            pp, w[:, 0:128], src[:, QC * f : QC * (f + 1)],
                            start=True, stop=False,
                        )
                        nc.tensor.matmul(
                            pp, w[:, 128:256], src[:, N + QC * f : N + QC * (f + 1)],
                            start=False, stop=True,
                        )
                        nc.scalar.copy(dst[:, QC * f : QC * (f + 1)], pp)
                # v projection, natural layout [seq, ch] into padded vhat
                for r in range(NKB):
                    vt = v_ps.tile([128, 32], F32, tag="v")
                    nc.tensor.matmul(
                        vt, xkv[:, 128 * r : 128 * (r + 1)], wv[:, 0:32],
                        start=True, stop=False,
                    )
                    nc.tensor.matmul(
                        vt, xkv[:, N + 128 * r : N + 128 * (r + 1)], wv[:, 32:64],
                        start=False, stop=True,
                    )
                    nc.vector.tensor_copy(vhat[:, VW * r : VW * r + 32], vt)
                # g projection -> tanh(0.5 x) (same ACT table set as exp)
                for f in range(4):
                    pp = proj_ps.tile([128, QC], F32, tag="pp")
                    nc.tensor.matmul(
                        pp, wg[:, 0:128], xq[:, QC * f : QC * (f + 1)],
                        start=True, stop=False,
                    )
                    nc.tensor.matmul(
                        pp, wg[:, 128:256], xq[:, N + QC * f : N + QC * (f + 1)],
                        start=False, stop=True,
                    )
                    nc.scalar.activation(
                        gt4[:, QC * f : QC * (f + 1)], pp, func=AF.Tanh, scale=0.5
                    )
                nc.vector.tensor_scalar_add(tp1[0:32, :], gt4[0:32, :], 1.0)

            # ---- stage 2+3: attention main loop ----
            with (
                tc.tile_pool(name="sc_ps", bufs=2, space="PSUM") as sc_pool,
                tc.tile_pool(name="o_ps", bufs=2, space="PSUM") as o_pool,
                tc.tile_pool(name="s3_ps", bufs=2, space="PSUM") as s3_pool,
                nc.named_scope("stage2_attn"),
            ):
                o_tiles = {}

                def stage3(c):
                    # gating + output projection for finished q-chunk c;
                    # emitted inside the next chunk to keep the PE queue fed.
                    og = ogpool.tile([128, QC], F16, tag="og")
                    nc.vector.memset(og[32:64, :], 0.0)
                    nc.vector.memset(og[64:96, :], 0.0)
                    nc.vector.tensor_mul(
                        og[0:33, :], o_tiles.pop(c)[0:33, :],
                        tp1[:, QC * c : QC * (c + 1)],
                    )
                    for j in range(4):
                        qb = 4 * c + j
                        s3 = s3_pool.tile([128, 257], F32, tag="s3")
                        nc.tensor.matmul(
                            s3, og[0:VW, 128 * j : 128 * (j + 1)], wo[0:VW, :],
                            start=True, stop=True,
                        )
                        nc.vector.tensor_copy(
                            outsb[:, 257 * qb : 257 * (qb + 1)], s3
                        )
                    nc.sync.dma_start(
                        out=out_d[:, 257 * 4 * c : 257 * 4 * (c + 1)],
                        in_=outsb[:, 257 * 4 * c : 257 * 4 * (c + 1)],
                    )

                for c in range(NQC):
                    o_ps = o_pool.tile([VW, QC], F32, tag="o")
                    o_tiles[c] = o_ps
                    for hg in range(HG):
                        s = HG * c + hg
                        ebs = ebsb[:, HW2 * s : HW2 * (s + 1)]
                        sc = sc_pool.tile([128, HW2], F32, tag="sc")
                        for i in range(2):
                            kb = 2 * hg + i
                            scs = sc[:, 512 * i : 512 * (i + 1)]
                            if _is_eye_block(hg, i):
                                nc.tensor.matmul(
                                    scs, eye,
                                    ebs[:, 512 * i : 512 * (i + 1)],
                                    start=True, stop=False,
                                )
                            nc.tensor.matmul(
                                scs,
                                kT4[:, 128 * kb : 128 * (kb + 1)],
                                qT4[:, QC * c : QC * (c + 1)],
                                start=not _is_eye_block(hg, i), stop=True,
                            )
                        if hg == 0 and c > 0:
                            stage3(c - 1)
                        pexp = pxpool.tile([128, HW2], F16, tag="pexp")
                        nc.scalar.activation(pexp, sc, func=AF.Exp, bias=nln16)
                        pt = pmpool.tile([128, HW2], F16, tag="p")
                        for i in range(2):
                            if not _is_eye_block(hg, i):
                                nc.vector.tensor_mul(
                                    pt[:, 512 * i : 512 * (i + 1)],
                                    pexp[:, 512 * i : 512 * (i + 1)],
                                    ebs[:, 512 * i : 512 * (i + 1)],
                                )
                        for i in range(2):
                            kb = 2 * hg + i
                            psrc = pexp if _is_eye_block(hg, i) else pt
                            nc.tensor.matmul(
                                o_ps,
                                vhat[:, VW * kb : VW * (kb + 1)],
                                psrc[:, 512 * i : 512 * (i + 1)],
                                start=(hg == 0 and i == 0),
                                stop=(hg == HG - 1 and i == 1),
                            )
                stage3(NQC - 1)

    nc.compile()
    return nc


def _get_nc():
    if "nc" not in _STATE:
        _STATE["nc"] = _build_nc()
    return _STATE["nc"]


def _pack2(m, dtype):
    """[256, X] -> [128, 2X]: c-chunk 0 in cols [0:X], chunk 1 in [X:2X]."""
    return np.ascontiguousarray(
        np.concatenate([m[0:128], m[128:256]], axis=1).astype(dtype)
    )


def _pad128(w):
    """[256, 32] head weight -> [256, 128] with cols 32:128 zero."""
    out = np.zeros((256, 128), dtype=np.float32)
    out[:, 0:32] = w
    return out


def kernel(q_x, kv_x, attn_bias, Wq, Wk, Wv, Wg, Wo):
    from concourse.bass_utils import run_bass_kernel_spmd

    BF = np.float16
    nc = _get_nc()

    q_x = np.asarray(q_x, dtype=np.float32)
    kv_x = np.asarray(kv_x, dtype=np.float32)
    attn_bias = np.asarray(attn_bias, dtype=np.float32)
    Wq = np.asarray(Wq, dtype=np.float32)
    Wk = np.asarray(Wk, dtype=np.float32)
    Wv = np.asarray(Wv, dtype=np.float32)
    Wg = np.asarray(Wg, dtype=np.float32)
    Wo = np.asarray(Wo, dtype=np.float32)

    xq = _pack2(np.ascontiguousarray(q_x[0].T), BF)
    xkv = _pack2(np.ascontiguousarray(kv_x[0].T), BF)
    eye = np.eye(128, dtype=BF)
    scale = np.float32(1.0 / np.sqrt(CH))

    in_maps = []
    for h in range(H):
        sl = slice(CH * h, CH * (h + 1))
        # 32 slabs [128, 1024], slab s=8c+hg covers q-chunk c, k-blocks
        # 2hg..2hg+1: slab[p, 512i+j] = bT[128*(2hg+i)+p, 512c+j].
        # PE-eye blocks keep raw bias; DVE blocks carry exp(bias).
        bT = attn_bias[0, h].T.astype(np.float32)  # [keys, queries]
        slabs = (
            bT.reshape(8, 2, 128, 4, 512)  # hg, i, p, c, j
            .transpose(3, 0, 2, 1, 4)  # c, hg, p, i, j
            .reshape(32, 128, 2, 512)
        ).copy()
        for s in range(32):
            for i in range(2):
                if not _is_eye_block(s % HG, i):
                    slabs[s, :, i] = np.exp(slabs[s, :, i])
        eb = np.ascontiguousarray(
            slabs.reshape(32, 128, HW2)
            .astype(BF)
            .transpose(1, 0, 2)
            .reshape(128, 32 * HW2)
        )
        woaug = np.zeros((128, 257), dtype=BF)
        woaug[0:32, 0:256] = (0.5 * Wo[sl, :]).astype(BF)
        woaug[32, 256] = 1.0
        in_maps.append(
            {
                "xq": xq,
                "xkv": xkv,
                "wq": _pack2(_pad128(Wq[:, sl] * scale), BF),
                "wk": _pack2(_pad128(Wk[:, sl]), BF),
                "wg": _pack2(_pad128(Wg[:, sl]), BF),
                "wv": _pack2(Wv[:, sl], BF),
                "wo": woaug,
                "eye": eye,
                "eb": eb,
            }
        )

    res = run_bass_kernel_spmd(nc, in_maps, list(range(H)))

    out = np.zeros((N, CQ), dtype=np.float32)
    for h in range(H):
        full = (
            res.results[h]["out"]
            .astype(np.float32)
            .reshape(128, 16, 257)
            .transpose(1, 0, 2)
            .reshape(N, 257)
        )
        out += full[:, 0:256] / full[:, 256][:, None]
    return out.reshape(B, N, CQ).astype(np.float32)
